# revision 1
# baseline (speedup 1.0000x reference)
"""GPTQ group-quantized linear (nn_GPTQLinear) on 8 Trainium2 NeuronCores.

out[b,s,o] = sum_k x[b,s,k] * (qweight[o,k] * scales[o, k//128]) + bias[o]

Full inputs in, full output out.  Sharding (internal): 4-way over batch rows
x 2-way over out_features -> per core M=2048 rows, N=2048 out feats, K=4096.

Per-core kernel:
  - qweight int32 [o,k] -> DVE dequant (x group scale, broadcast along free) ->
    bf16 -> PE transpose -> resident wT [128, K/128, N] bf16 in SBUF.
  - x fp32 [m,k] -> SWDGE cast DMA -> bf16 DRAM staging -> HWDGE DMA-transpose
    -> xT chunks [128, K/128, M_SC] bf16.
  - bf16 matmuls (lhsT = xT slice, rhs = wT slice), fp32 accumulate in PSUM
    over K, bias added from a broadcast SBUF tile in the epilogue.
"""

from contextlib import ExitStack

import numpy as np

import concourse.bass as bass
import concourse.bacc as bacc
import concourse.mybir as mybir
import concourse.tile as tile
from concourse import bass_utils
from concourse.masks import make_identity

F32 = mybir.dt.float32
BF16 = mybir.dt.bfloat16
I32 = mybir.dt.int32

P = 128            # partitions = k-tile = quant group size
N_CH = 512         # out-feature chunk (one PSUM bank of fp32)
M_SC = 256         # x rows per DMA-transpose super-chunk
Q_SLAB_K = 2048    # k extent of one qweight load
DQ_CHUNK = 512     # k extent of one dequant/transpose chunk (4 k-tiles)

# full problem / sharding constants (hardcoded per harness contract)
B, S, K_FULL, NF = 4, 2048, 4096, 4096
MB_SHARDS, NB_SHARDS = 4, 2
M_CORE, N_CORE = (B * S) // MB_SHARDS, NF // NB_SHARDS
N_CORES = 8


def emit(tc, ctx, o_ap, x_ap, q_ap, s_ap, b_ap):
    nc = tc.nc
    M, K = x_ap.shape
    N = q_ap.shape[0]
    KT = K // P
    NCH = N // N_CH
    NSC = M // M_SC
    MT = M_SC // P
    q_slab_k = min(Q_SLAB_K, K)
    OC_PER_CH = N_CH // P

    const = ctx.enter_context(tc.tile_pool(name="const", bufs=1))
    wt_pool = ctx.enter_context(tc.tile_pool(name="wt", bufs=1))
    pan_pool = ctx.enter_context(tc.tile_pool(name="pan", bufs=KT, space="DRAM"))
    qs_pool = ctx.enter_context(tc.tile_pool(name="qs", bufs=2))
    wdq_pool = ctx.enter_context(tc.tile_pool(name="wdq", bufs=4))
    xt_pool = ctx.enter_context(tc.tile_pool(name="xt", bufs=2))
    out_pool = ctx.enter_context(tc.tile_pool(name="outp", bufs=3))
    pst_pool = ctx.enter_context(tc.tile_pool(name="pst", bufs=2, space="PSUM"))
    psmm_pool = ctx.enter_context(tc.tile_pool(name="psmm", bufs=4, space="PSUM"))

    # ---- constants ----
    identity = const.tile([P, P], BF16, tag="identity")
    make_identity(nc, identity[:])
    scales_sb = const.tile([P, N // P, KT], F32, tag="scales")
    nc.scalar.dma_start(scales_sb[:], s_ap.rearrange("(oc p) g -> p oc g", p=P))
    bias_sb = const.tile([1, N], F32, tag="bias")
    nc.scalar.dma_start(bias_sb[:], b_ap[None, :])
    ones = const.tile([1, P], F32, tag="ones")
    nc.vector.memset(ones[:], 1.0)

    # bias broadcast to all 128 partitions via a K=1 fp32 matmul
    bias_bc = const.tile([P, N], F32, tag="bias_bc")
    for n in range(NCH):
        psb = psmm_pool.tile([P, N_CH], F32, bufs=1)
        nc.tensor.matmul(
            psb[:], ones[:], bias_sb[:, n * N_CH : (n + 1) * N_CH],
            start=True, stop=True,
        )
        nc.vector.tensor_copy(bias_bc[:, n * N_CH : (n + 1) * N_CH], psb[:])

    # ---- x: fp32 -> bf16 k-panels [M, 128] in DRAM (SWDGE cast DMA).
    # Panels are contiguous so the later DMA-transposes read at line rate.
    panels = []
    for kt in range(KT):
        t = pan_pool.tile([M, P], BF16, tag=f"pan{kt}", name=f"pan{kt}")
        nc.gpsimd.dma_start(t[:], x_ap[:, kt * P : (kt + 1) * P])
        panels.append(t)

    wt = [
        wt_pool.tile([P, KT, N_CH], BF16, tag=f"wt{n}", name=f"wt{n}")
        for n in range(NCH)
    ]

    def dequant_group(n_ch):
        """Dequantize o-slabs for out-feature chunk n_ch into wt[n_ch]."""
        for oci in range(OC_PER_CH):
            oc = n_ch * OC_PER_CH + oci
            o_col = oci * P
            for kh in range(K // q_slab_k):
                qt = qs_pool.tile([P, q_slab_k], I32, name="qt")
                nc.scalar.dma_start(
                    qt[:],
                    q_ap[oc * P : (oc + 1) * P, kh * q_slab_k : (kh + 1) * q_slab_k],
                )
                for cc in range(q_slab_k // DQ_CHUNK):
                    kt0 = (kh * q_slab_k + cc * DQ_CHUNK) // P
                    g = DQ_CHUNK // P  # k-tiles (= groups) per chunk
                    wdq = wdq_pool.tile([P, DQ_CHUNK], BF16, name="wdq")
                    nc.vector.tensor_tensor(
                        wdq[:].rearrange("p (g i) -> p g i", i=P),
                        qt[:, cc * DQ_CHUNK : (cc + 1) * DQ_CHUNK].rearrange(
                            "p (g i) -> p g i", i=P
                        ),
                        scales_sb[:, oc, kt0 : kt0 + g, None].to_broadcast([P, g, P]),
                        mybir.AluOpType.mult,
                    )
                    ps = pst_pool.tile([P, DQ_CHUNK], BF16, name="ps_t")
                    for j in range(g):
                        nc.tensor.transpose(
                            ps[:, j * P : (j + 1) * P], wdq[:, j * P : (j + 1) * P],
                            identity[:],
                        )
                    nc.vector.tensor_copy(
                        wt[n_ch][:, kt0 : kt0 + g, o_col : o_col + P],
                        ps[:].rearrange("p (g i) -> p g i", i=P),
                    )

    def load_xt(sc):
        xt = xt_pool.tile([P, KT, M_SC], BF16, name="xt")
        for kt in range(KT):
            nc.sync.dma_start(
                xt[:, kt, :],
                panels[kt][sc * M_SC : (sc + 1) * M_SC, :],
                transpose=True,
            )
        return xt

    def mm_block(xt, sc, n):
        for mt in range(MT):
            ps = psmm_pool.tile([P, N_CH], F32, name="ps_mm")
            for kt in range(KT):
                nc.tensor.matmul(
                    ps[:],
                    xt[:, kt, mt * P : (mt + 1) * P],
                    wt[n][:, kt, :],
                    start=(kt == 0),
                    stop=(kt == KT - 1),
                )
            ot = out_pool.tile([P, N_CH], F32, name="ot")
            nc.vector.tensor_tensor(
                ot[:], ps[:], bias_bc[:, n * N_CH : (n + 1) * N_CH],
                mybir.AluOpType.add,
            )
            m0 = sc * M_SC + mt * P
            nc.gpsimd.dma_start(
                o_ap[m0 : m0 + P, n * N_CH : (n + 1) * N_CH], ot[:]
            )

    # ---- interleave: dequant group g, then matmuls of (sc0, n=g) so the
    # PE instruction stream alternates transpose bursts with matmul bursts.
    xt0 = None
    for g in range(NCH):
        dequant_group(g)
        if g == 0:
            xt0 = load_xt(0)
        mm_block(xt0, 0, g)
    for sc in range(1, NSC):
        xt = load_xt(sc)
        for n in range(NCH):
            mm_block(xt, sc, n)


def build_program(M=M_CORE, N=N_CORE, K=K_FULL):
    nc = bacc.Bacc("TRN2", target_bir_lowering=False, debug=False)
    x = nc.dram_tensor("x", [M, K], F32, kind="ExternalInput")
    q = nc.dram_tensor("qweight", [N, K], I32, kind="ExternalInput")
    s = nc.dram_tensor("scales", [N, K // P], F32, kind="ExternalInput")
    b = nc.dram_tensor("bias", [N], F32, kind="ExternalInput")
    o = nc.dram_tensor("out", [M, N], F32, kind="ExternalOutput")
    with tile.TileContext(nc) as tc:
        with ExitStack() as ctx:
            emit(tc, ctx, o.ap(), x.ap(), q.ap(), s.ap(), b.ap())
    nc.compile()
    return nc


def enable_ntff_profiling():
    """Register the axon NTFF profile hook (the image's antenv lacks
    axon_hooks, so trn_boot degrades silently).  Returns True on success."""
    import sys
    import types

    try:
        from antenv.axon_hooks import get_axon_ntff_profile_hook  # noqa: F401

        return True
    except ImportError:
        pass
    try:
        from trn_agent_boot.trn_boot import _ntff_profile_via_ctypes

        hook = _ntff_profile_via_ctypes("/opt/axon/libaxon_pjrt.so")
        if hook is None:
            return False
        mod = types.ModuleType("antenv.axon_hooks")
        mod._hook = hook

        def set_axon_ntff_profile_hook(h):
            mod._hook = h

        def get_axon_ntff_profile_hook():
            return mod._hook

        mod.set_axon_ntff_profile_hook = set_axon_ntff_profile_hook
        mod.get_axon_ntff_profile_hook = get_axon_ntff_profile_hook
        sys.modules["antenv.axon_hooks"] = mod
        return True
    except Exception:
        return False


_CACHE = {}


def _get_program():
    if "nc" not in _CACHE:
        _CACHE["nc"] = build_program()
    return _CACHE["nc"]


def _shard_inputs(x, qweight, scales, bias):
    x2 = np.asarray(x, dtype=np.float32).reshape(B * S, K_FULL)
    qweight = np.asarray(qweight, dtype=np.int32)
    scales = np.asarray(scales, dtype=np.float32)
    bias = np.asarray(bias, dtype=np.float32)
    in_maps = []
    for c in range(N_CORES):
        mb, nb = divmod(c, NB_SHARDS)
        in_maps.append(
            {
                "x": np.ascontiguousarray(x2[mb * M_CORE : (mb + 1) * M_CORE]),
                "qweight": np.ascontiguousarray(
                    qweight[nb * N_CORE : (nb + 1) * N_CORE]
                ),
                "scales": np.ascontiguousarray(
                    scales[nb * N_CORE : (nb + 1) * N_CORE]
                ),
                "bias": np.ascontiguousarray(bias[nb * N_CORE : (nb + 1) * N_CORE]),
            }
        )
    return in_maps


def _gather_output(results):
    out = np.empty((B * S, NF), dtype=np.float32)
    for c in range(N_CORES):
        mb, nb = divmod(c, NB_SHARDS)
        out[mb * M_CORE : (mb + 1) * M_CORE, nb * N_CORE : (nb + 1) * N_CORE] = (
            results[c]["out"]
        )
    return out.reshape(B, S, NF)


def run_sharded(x, qweight, scales, bias, **spmd_kwargs):
    """Run on all 8 cores; returns (full_output, BassKernelResults)."""
    if spmd_kwargs.get("trace"):
        enable_ntff_profiling()
    nc = _get_program()
    in_maps = _shard_inputs(x, qweight, scales, bias)
    res = bass_utils.run_bass_kernel_spmd(
        nc, in_maps, core_ids=list(range(N_CORES)), **spmd_kwargs
    )
    return _gather_output(res.results), res


def kernel(x, qweight, scales, bias):
    out, _ = run_sharded(x, qweight, scales, bias)
    return out



# revision 10
# speedup vs baseline: 1.1313x; 1.1313x over previous
"""GPTQ group-quantized linear (nn_GPTQLinear) on 8 Trainium2 NeuronCores.

out[b,s,o] = sum_k x[b,s,k] * (qweight[o,k] * scales[o, k//128]) + bias[o]

Full inputs in, full output out.  Sharding (internal): 4-way over batch rows
x 2-way over out_features -> per core M=2048 rows, N=2048 out feats, K=4096.

Per-core kernel:
  - qweight int32 [o,k] -> DVE dequant (x group scale, broadcast along free) ->
    bf16 -> PE transpose -> resident wT [128, K/128, N] bf16 in SBUF.
  - x fp32 [m,k] -> SWDGE cast DMA (contiguous row chunks) -> bf16 DRAM
    staging -> HWDGE DMA-transpose (split across sync+scalar queues)
    -> xT chunks [128, K/128, M_SC] bf16.
  - bf16 matmuls (lhsT = xT slice, rhs = wT slice), fp32 accumulate in PSUM
    over K, bias added from a broadcast SBUF tile in the epilogue.
"""

from contextlib import ExitStack

import numpy as np

import concourse.bass as bass
import concourse.bacc as bacc
import concourse.mybir as mybir
import concourse.tile as tile
from concourse import bass_utils
from concourse.masks import make_identity

F32 = mybir.dt.float32
BF16 = mybir.dt.bfloat16
I32 = mybir.dt.int32

P = 128            # partitions = k-tile = quant group size
N_CH = 512         # out-feature chunk (one PSUM bank of fp32)
M_SC = 256         # x rows per DMA-transpose super-chunk
Q_SLAB_K = 2048    # k extent of one qweight load
DQ_CHUNK = 512     # k extent of one dequant/transpose chunk (4 k-tiles)

# full problem / sharding constants (hardcoded per harness contract)
B, S, K_FULL, NF = 4, 2048, 4096, 4096
MB_SHARDS, NB_SHARDS = 4, 2
M_CORE, N_CORE = (B * S) // MB_SHARDS, NF // NB_SHARDS
N_CORES = 8


def emit(tc, ctx, o_ap, x_ap, q_ap, s_ap, b_ap):
    nc = tc.nc
    M, K = x_ap.shape
    N = q_ap.shape[0]
    KT = K // P
    NCH = N // N_CH
    NSC = M // M_SC
    MT = M_SC // P
    q_slab_k = min(Q_SLAB_K, K)
    OC_PER_CH = N_CH // P

    const = ctx.enter_context(tc.tile_pool(name="const", bufs=1))
    wt_pool = ctx.enter_context(tc.tile_pool(name="wt", bufs=1))
    pan_pool = ctx.enter_context(tc.tile_pool(name="pan", bufs=8, space="DRAM"))
    qs_pool = ctx.enter_context(tc.tile_pool(name="qs", bufs=2))
    wdq_pool = ctx.enter_context(tc.tile_pool(name="wdq", bufs=4))
    xt_pool = ctx.enter_context(tc.tile_pool(name="xt", bufs=2))
    out_pool = ctx.enter_context(tc.tile_pool(name="outp", bufs=3))
    pst_pool = ctx.enter_context(tc.tile_pool(name="pst", bufs=2, space="PSUM"))
    psmm_pool = ctx.enter_context(tc.tile_pool(name="psmm", bufs=4, space="PSUM"))

    # ---- constants ----
    identity = const.tile([P, P], BF16, tag="identity")
    make_identity(nc, identity[:])
    scales_sb = const.tile([P, N // P, KT], F32, tag="scales")
    nc.scalar.dma_start(scales_sb[:], s_ap.rearrange("(oc p) g -> p oc g", p=P))
    bias_sb = const.tile([1, N], F32, tag="bias")
    nc.scalar.dma_start(bias_sb[:], b_ap[None, :])
    ones = const.tile([1, P], F32, tag="ones")
    nc.vector.memset(ones[:], 1.0)

    # bias broadcast to all 128 partitions via a K=1 fp32 matmul
    bias_bc = const.tile([P, N], F32, tag="bias_bc")
    for n in range(NCH):
        psb = psmm_pool.tile([P, N_CH], F32, bufs=1)
        nc.tensor.matmul(
            psb[:], ones[:], bias_sb[:, n * N_CH : (n + 1) * N_CH],
            start=True, stop=True,
        )
        nc.vector.tensor_copy(bias_bc[:, n * N_CH : (n + 1) * N_CH], psb[:])

    # ---- x: fp32 -> bf16 row chunks [M_SC, K] in DRAM (SWDGE cast DMA).
    # Contiguous on both sides -> line-rate cast; one chunk per super-chunk
    # so the first DMA-transposes start after ~1 chunk instead of all of x.
    xbf = []
    for sc in range(NSC):
        t = pan_pool.tile([M_SC, K], BF16, tag=f"xbf{sc}", name=f"xbf{sc}")
        nc.gpsimd.dma_start(t[:], x_ap[sc * M_SC : (sc + 1) * M_SC, :])
        xbf.append(t)

    wt = [
        wt_pool.tile([P, KT, N_CH], BF16, tag=f"wt{n}", name=f"wt{n}")
        for n in range(NCH)
    ]

    def dequant_group(n_ch):
        """Dequantize o-slabs for out-feature chunk n_ch into wt[n_ch]."""
        for oci in range(OC_PER_CH):
            oc = n_ch * OC_PER_CH + oci
            o_col = oci * P
            for kh in range(K // q_slab_k):
                qt = qs_pool.tile([P, q_slab_k], I32, name="qt")
                nc.scalar.dma_start(
                    qt[:],
                    q_ap[oc * P : (oc + 1) * P, kh * q_slab_k : (kh + 1) * q_slab_k],
                )
                for cc in range(q_slab_k // DQ_CHUNK):
                    kt0 = (kh * q_slab_k + cc * DQ_CHUNK) // P
                    g = DQ_CHUNK // P  # k-tiles (= groups) per chunk
                    wdq = wdq_pool.tile([P, DQ_CHUNK], BF16, name="wdq")
                    nc.vector.tensor_tensor(
                        wdq[:].rearrange("p (g i) -> p g i", i=P),
                        qt[:, cc * DQ_CHUNK : (cc + 1) * DQ_CHUNK].rearrange(
                            "p (g i) -> p g i", i=P
                        ),
                        scales_sb[:, oc, kt0 : kt0 + g, None].to_broadcast([P, g, P]),
                        mybir.AluOpType.mult,
                    )
                    ps = pst_pool.tile([P, DQ_CHUNK], BF16, name="ps_t")
                    for j in range(g):
                        nc.tensor.transpose(
                            ps[:, j * P : (j + 1) * P], wdq[:, j * P : (j + 1) * P],
                            identity[:],
                        )
                    nc.vector.tensor_copy(
                        wt[n_ch][:, kt0 : kt0 + g, o_col : o_col + P],
                        ps[:].rearrange("p (g i) -> p g i", i=P),
                    )

    def load_xt(sc):
        xt = xt_pool.tile([P, KT, M_SC], BF16, name="xt")
        for kt in range(KT):
            nc.sync.dma_start(
                xt[:, kt, :],
                xbf[sc][:, kt * P : (kt + 1) * P],
                transpose=True,
            )
        return xt

    def mm_block(xt, sc, n):
        for mt in range(MT):
            ps = psmm_pool.tile([P, N_CH], F32, name="ps_mm")
            for kt in range(KT):
                nc.tensor.matmul(
                    ps[:],
                    xt[:, kt, mt * P : (mt + 1) * P],
                    wt[n][:, kt, :],
                    start=(kt == 0),
                    stop=(kt == KT - 1),
                )
            ot = out_pool.tile([P, N_CH], F32, name="ot")
            nc.vector.tensor_tensor(
                ot[:], ps[:], bias_bc[:, n * N_CH : (n + 1) * N_CH],
                mybir.AluOpType.add,
            )
            m0 = sc * M_SC + mt * P
            nc.gpsimd.dma_start(
                o_ap[m0 : m0 + P, n * N_CH : (n + 1) * N_CH], ot[:]
            )

    # ---- interleave: dequant group g, then matmuls of (sc0, n=g) so the
    # PE instruction stream alternates transpose bursts with matmul bursts.
    xt0 = None
    for g in range(NCH):
        dequant_group(g)
        if g == 0:
            xt0 = load_xt(0)
        mm_block(xt0, 0, g)
    for sc in range(1, NSC):
        xt = load_xt(sc)
        for n in range(NCH):
            mm_block(xt, sc, n)


def build_program(M=M_CORE, N=N_CORE, K=K_FULL):
    nc = bacc.Bacc("TRN2", target_bir_lowering=False, debug=False)
    x = nc.dram_tensor("x", [M, K], F32, kind="ExternalInput")
    q = nc.dram_tensor("qweight", [N, K], I32, kind="ExternalInput")
    s = nc.dram_tensor("scales", [N, K // P], F32, kind="ExternalInput")
    b = nc.dram_tensor("bias", [N], F32, kind="ExternalInput")
    o = nc.dram_tensor("out", [M, N], F32, kind="ExternalOutput")
    with tile.TileContext(nc) as tc:
        with ExitStack() as ctx:
            emit(tc, ctx, o.ap(), x.ap(), q.ap(), s.ap(), b.ap())
    nc.compile()
    return nc


def enable_ntff_profiling():
    """Register the axon NTFF profile hook (the image's antenv lacks
    axon_hooks, so trn_boot degrades silently).  Returns True on success."""
    import sys
    import types

    try:
        from antenv.axon_hooks import get_axon_ntff_profile_hook  # noqa: F401

        return True
    except ImportError:
        pass
    try:
        from trn_agent_boot.trn_boot import _ntff_profile_via_ctypes

        hook = _ntff_profile_via_ctypes("/opt/axon/libaxon_pjrt.so")
        if hook is None:
            return False
        mod = types.ModuleType("antenv.axon_hooks")
        mod._hook = hook

        def set_axon_ntff_profile_hook(h):
            mod._hook = h

        def get_axon_ntff_profile_hook():
            return mod._hook

        mod.set_axon_ntff_profile_hook = set_axon_ntff_profile_hook
        mod.get_axon_ntff_profile_hook = get_axon_ntff_profile_hook
        sys.modules["antenv.axon_hooks"] = mod
        return True
    except Exception:
        return False


_CACHE = {}


def _get_program():
    if "nc" not in _CACHE:
        _CACHE["nc"] = build_program()
    return _CACHE["nc"]


def _shard_inputs(x, qweight, scales, bias):
    x2 = np.asarray(x, dtype=np.float32).reshape(B * S, K_FULL)
    qweight = np.asarray(qweight, dtype=np.int32)
    scales = np.asarray(scales, dtype=np.float32)
    bias = np.asarray(bias, dtype=np.float32)
    in_maps = []
    for c in range(N_CORES):
        mb, nb = divmod(c, NB_SHARDS)
        in_maps.append(
            {
                "x": np.ascontiguousarray(x2[mb * M_CORE : (mb + 1) * M_CORE]),
                "qweight": np.ascontiguousarray(
                    qweight[nb * N_CORE : (nb + 1) * N_CORE]
                ),
                "scales": np.ascontiguousarray(
                    scales[nb * N_CORE : (nb + 1) * N_CORE]
                ),
                "bias": np.ascontiguousarray(bias[nb * N_CORE : (nb + 1) * N_CORE]),
            }
        )
    return in_maps


def _gather_output(results):
    out = np.empty((B * S, NF), dtype=np.float32)
    for c in range(N_CORES):
        mb, nb = divmod(c, NB_SHARDS)
        out[mb * M_CORE : (mb + 1) * M_CORE, nb * N_CORE : (nb + 1) * N_CORE] = (
            results[c]["out"]
        )
    return out.reshape(B, S, NF)


def run_sharded(x, qweight, scales, bias, **spmd_kwargs):
    """Run on all 8 cores; returns (full_output, BassKernelResults)."""
    if spmd_kwargs.get("trace"):
        enable_ntff_profiling()
    nc = _get_program()
    in_maps = _shard_inputs(x, qweight, scales, bias)
    res = bass_utils.run_bass_kernel_spmd(
        nc, in_maps, core_ids=list(range(N_CORES)), **spmd_kwargs
    )
    return _gather_output(res.results), res


def kernel(x, qweight, scales, bias):
    out, _ = run_sharded(x, qweight, scales, bias)
    return out

